# revision 1
# baseline (speedup 1.0000x reference)
"""Distributed GCNConv kernel for Trainium2 (8 NeuronCores).

Graph-partition (expert-style) sharding by destination node: core k owns
destination rows [k*num_owned/8, (k+1)*num_owned/8). Edges whose dst lies
outside every core's owned range (row >= num_owned) are dropped (the
reference discards those aggregates).

Per core (SPMD, one NEFF):
  phase A: h = (deg * x) @ W for ALL nodes, replicated (for a random
    graph the halo is ~everything, so recomputing the 3.3 GFLOP matmul
    beats halo exchange). h is stored bf16 in a partition-major HBM table
    (row of node n = (n%128)*TA + n//128) so the store is one contiguous
    run per partition.
  phase B: edges sorted by (dst supertile, 32K source window, dst tile);
    per-(tile,window) slot counts are padded only to the max across cores
    (SPMD uniformity). gpsimd.dma_gather (int16 window-relative indices,
    <=2048/call, single_packet=False - larger calls overflow the SWDGE
    ring) pulls 256B messages; DVE builds a one-hot S per 128-edge chunk
    via tensor_scalar((iota + 128*v) == rowloc) where v selects the dst
    tile for chunks that span tile boundaries; PE accumulates
    psum[dst,feat] += S^T @ msgs with one PSUM bank per dst tile (the HW
    zeroes accumulation state at bank granularity); a final DVE pass
    applies deg_dst (+bias) and stores partition-major (unpermuted on
    host).

Self-contained: only needs numpy / ml_dtypes / the concourse Bass stack.
"""

import numpy as np
import ml_dtypes

import concourse.bacc as bacc
import concourse.bass as bass
import concourse.mybir as mybir
import concourse.tile as tile
from concourse.bass_utils import run_bass_kernel_spmd

P = 128
N_CORES = 8
WIN = 32768  # int16 gather-index window (dma_gather idxs are int16)
ST = 8       # dst tiles per supertile (one PSUM bank per tile, 8 banks)
GA = 32      # node tiles per phase-A block (1 MiB DMAs)
GMAX = 2048  # max gather indices per dma_gather call (SWDGE ring limit)
BF16 = mybir.dt.bfloat16
F32 = mybir.dt.float32
I16 = mybir.dt.int16
npbf16 = ml_dtypes.bfloat16
PAD_ROWLOC = -1.0  # one-hot miss marker (never equals iota + 128*v >= 0)


def _wrap_idx(a):
    """dma_gather index layout: linear index i lives at [i%16, i//16],
    replicated across the 8 Q7 cores -> [128, len//16]."""
    return np.tile(a.reshape(-1, 16).T, (8, 1))


def _plan(row, col, n_local, n_owned):
    """Host-side graph partitioning. Returns the SPMD-uniform schedule and
    the per-core packed index/rowloc arrays."""
    own = n_owned // N_CORES
    T = -(-own // P)                    # dst tiles per core
    S = -(-T // ST)                     # supertiles per core
    NW = -(-n_local // WIN)             # gather windows
    n_pad = -(-n_local // P) * P        # padded node count (phase A)

    TA = n_pad // P

    row = np.asarray(row).astype(np.int64)
    col = np.asarray(col).astype(np.int64)
    keep = row < n_owned
    r, c = row[keep], col[keep]
    core = r // own
    rl = r - core * own
    t = rl // P
    # the h table is stored partition-major (table row of node n is
    # (n%128)*TA + n//128) so the phase-A store is one long contiguous
    # run per partition; gather indices use table rows
    c = (c % P) * TA + (c // P)
    w = c // WIN

    # slot counts per (core, t, w); pad only to the max across cores (the
    # program must be SPMD-uniform). Runs pack back-to-back UNALIGNED; a
    # 128-edge chunk that spans tile boundaries issues one matmul per
    # covered tile, with the one-hot built against an offset iota slice.
    key = (core * T + t) * NW + w
    counts = np.bincount(key, minlength=N_CORES * T * NW).reshape(N_CORES, T, NW)
    C = counts.max(axis=0).astype(np.int64)  # [T, NW]
    assert counts.sum(axis=(0, 2)).min() > 0, "empty dst tile"

    # run order: supertile-major, then window, then tile (so each (s,w) is
    # one contiguous same-window gather call)
    runs = []   # (s, w, t, slot_off)
    calls = []  # (s, w, slot_off, L, tail) per (s,w), L 128-aligned; the
                # last `tail` slots are common pad (gathered as idx -1,
                # which the DMA skips)
    jobs = {}   # (s, w) -> list of (chunk_local, tile, iota_variant)
    first_job = {}  # tile -> (w, chunk_local, tile)
    last_job = {}
    off = 0
    t0_of_chunk = []  # global chunk -> first covered tile
    for s in range(S):
        ts = range(s * ST, min((s + 1) * ST, T))
        for wi in range(NW):
            call_off = off
            run_list = []
            for ti in ts:
                if C[ti, wi]:
                    runs.append((s, wi, ti, off))
                    run_list.append((ti, off, int(C[ti, wi])))
                    off += int(C[ti, wi])
            off_real = off
            L = -(-(off - call_off) // P) * P
            if not L:
                continue
            off = call_off + L
            calls.append((s, wi, call_off, L, call_off + L - off_real))
            nchk = L // P
            t0 = [None] * nchk
            jlist = []
            for (ti, o_r, cnt) in run_list:
                k_first = (o_r - call_off) // P
                k_last = (o_r + cnt - 1 - call_off) // P
                for k in range(k_first, k_last + 1):
                    if t0[k] is None:
                        t0[k] = ti
                    jlist.append((k, ti))
            jlist.sort()
            jlist2 = []
            for (k, ti) in jlist:
                v = ti - t0[k]
                assert 0 <= v < 64, (ti, t0[k])
                jlist2.append((k, ti, v))
                jk = (wi, k, ti)
                if ti not in first_job:
                    first_job[ti] = jk
                last_job[ti] = jk
            jobs[(s, wi)] = jlist2
            t0_of_chunk.extend(ti if ti is not None else 0 for ti in t0)
    TOT = off
    NCHK = TOT // P
    T0g = np.asarray(t0_of_chunk, dtype=np.int64)
    assert len(T0g) == NCHK

    # per-core slot packing
    per_core = []
    for k in range(N_CORES):
        m = core == k
        ck, rlk, tk, wk = c[m], rl[m], t[m], w[m]
        # composite key in stream order: (supertile, window, tile)
        g = (tk // ST) * (NW * T) + wk * T + tk
        order = np.argsort(g, kind="stable")
        ck, rlk, tk, wk, g = ck[order], rlk[order], tk[order], wk[order], g[order]
        ngrp = S * NW * T
        cnt = np.bincount(g, minlength=ngrp)
        starts = np.concatenate(([0], np.cumsum(cnt)))[:-1]
        rank = np.arange(len(g)) - starts[g]
        # map stream-order key -> run slot offset
        lut = np.full(ngrp, -1, dtype=np.int64)
        for (si, wi, ti, o) in runs:
            lut[si * (NW * T) + wi * T + ti] = o
        dest = lut[g] + rank
        assert dest.min() >= 0

        gidx = np.zeros(TOT, dtype=np.int16)
        rloc = np.full(TOT, PAD_ROWLOC, dtype=np.float32)
        gidx[dest] = (ck - wk * WIN).astype(np.int16)
        # rowloc relative to the chunk's FIRST covered tile: tiles later in
        # the chunk land at +128/+256 (matched by the offset iota slices)
        rloc[dest] = (rlk - tk * P + P * (tk - T0g[dest // P])).astype(np.float32)

        idx_wrapped = np.concatenate(
            [_wrap_idx(gidx[o:o + L]) for (_, _, o, L, _) in calls], axis=1
        )
        RL = np.ascontiguousarray(rloc.reshape(NCHK, P).T)
        per_core.append((idx_wrapped, RL))

    plan = dict(
        n_local=n_local, n_owned=n_owned, own=own, T=T, S=S, NW=NW,
        n_pad=n_pad, C=C, runs=runs, calls=calls, jobs=jobs, TOT=TOT,
        NCHK=NCHK, first_job=first_job, last_job=last_job,
    )
    return plan, per_core


def _build(plan, bias_zero=False):
    """Emit the Bass/Tile program (identical for all cores)."""
    n_pad, T, S, NW = plan["n_pad"], plan["T"], plan["S"], plan["NW"]
    C, calls, jobs = plan["C"], plan["calls"], plan["jobs"]
    TOT, NCHK = plan["TOT"], plan["NCHK"]
    first_job, last_job = plan["first_job"], plan["last_job"]
    TA = n_pad // P  # phase-A node tiles

    nc = bacc.Bacc("TRN2", target_bir_lowering=False, debug=False,
                   enable_asserts=False, num_devices=N_CORES)

    xt = nc.dram_tensor("xt", [P, n_pad], BF16, kind="ExternalInput")
    wgt = nc.dram_tensor("wgt", [P, P], BF16, kind="ExternalInput")
    dega = nc.dram_tensor("dega", [P, TA], F32, kind="ExternalInput")
    iot = nc.dram_tensor("iot", [P, P], BF16, kind="ExternalInput")
    biasb = nc.dram_tensor("biasb", [P, P], F32, kind="ExternalInput")
    idx = nc.dram_tensor("idx", [P, TOT // 16], I16, kind="ExternalInput")
    rld = nc.dram_tensor("rl", [P, NCHK], F32, kind="ExternalInput")
    degd = nc.dram_tensor("degd", [P, T], F32, kind="ExternalInput")
    out = nc.dram_tensor("out", [T * P, P], F32, kind="ExternalOutput")
    hst = nc.dram_tensor("h_stash", [n_pad, P], BF16, kind="Internal")

    with tile.TileContext(nc) as tc:
        with (
            tc.tile_pool(name="const", bufs=1) as constp,
            tc.tile_pool(name="xtp", bufs=2) as xtp,
            tc.tile_pool(name="hsb", bufs=2) as hsb,
            # one shared PSUM pool: every tile is one full bank (the HW
            # zeroes accumulation state at 2KB-bank granularity, so each
            # accumulator group must own its bank)
            tc.tile_pool(name="ps", bufs=8, space="PSUM") as psp,
            tc.tile_pool(name="msgs", bufs=4) as msgsp,
            tc.tile_pool(name="sone", bufs=12) as sonep,
            tc.tile_pool(name="osb", bufs=2) as osb,
        ):
            # resident constants / metadata
            w_sb = constp.tile([P, P], BF16)
            nc.sync.dma_start(w_sb[:], wgt[:, :])
            dega_sb = constp.tile([P, TA], F32)
            nc.sync.dma_start(dega_sb[:], dega[:, :])
            iot_sb = constp.tile([P, P], BF16)
            nc.sync.dma_start(iot_sb[:], iot[:, :])
            bias_sb = constp.tile([P, P], F32)
            nc.sync.dma_start(bias_sb[:], biasb[:, :])
            idx_sb = constp.tile([P, TOT // 16], I16)
            nc.sync.dma_start(idx_sb[:], idx[:, :])
            rl_sb = constp.tile([P, NCHK], F32)
            nc.sync.dma_start(rl_sb[:], rld[:, :])
            degd_sb = constp.tile([P, T], F32)
            nc.sync.dma_start(degd_sb[:], degd[:, :])

            # ---- phase A: h = (deg * x) @ W, stored bf16 node-major ----
            for b0 in range(0, TA, GA):
                nt = min(GA, TA - b0)
                xt_t = xtp.tile([P, GA * P], BF16, tag="xt")
                nc.sync.dma_start(xt_t[:, :nt * P],
                                  xt[:, b0 * P:(b0 + nt) * P])
                h_t = hsb.tile([P, GA * P], BF16, tag="h")
                for j0 in range(0, nt, 4):
                    nj = min(4, nt - j0)
                    ps = psp.tile([P, 512], F32, tag="ps")
                    for cc in range(nj):
                        nc.tensor.matmul(
                            ps[:, cc * P:(cc + 1) * P],
                            xt_t[:, (j0 + cc) * P:(j0 + cc + 1) * P],
                            w_sb[:],
                        )
                    for cc in range(nj):
                        tcol = b0 + j0 + cc
                        if cc % 2 == 0:
                            nc.vector.tensor_scalar(
                                h_t[:, (j0 + cc) * P:(j0 + cc + 1) * P],
                                ps[:, cc * P:(cc + 1) * P],
                                dega_sb[:, tcol:tcol + 1], None,
                                mybir.AluOpType.mult,
                            )
                        else:
                            nc.scalar.activation(
                                h_t[:, (j0 + cc) * P:(j0 + cc + 1) * P],
                                ps[:, cc * P:(cc + 1) * P],
                                mybir.ActivationFunctionType.Copy,
                                scale=dega_sb[:, tcol:tcol + 1],
                            )
                hv = hst[:, :].rearrange("(p j) f -> p j f", p=P)[:, b0:b0 + nt, :]
                nc.sync.dma_start(
                    hv, h_t[:, :nt * P].rearrange("p (j f) -> p j f", f=P))

            # ---- phase B: gather + one-hot matmul segment-sum ----
            max_chunks = max(L for (_, _, _, L, _) in calls) // P

            for s in range(S):
                ts0 = s * ST
                nts = min(ST, T - ts0)
                # one PSUM bank per dst tile; the group stays open across
                # all gather windows of this supertile
                pbs = [psp.tile([P, P], F32, tag="ps", name=f"pb{s}_{i}")
                       for i in range(nts)]
                for (cs, wi, o, L, tail) in calls:
                    if cs != s:
                        continue
                    nchk = L // P
                    wbase = wi * WIN
                    wsz = min(WIN, n_pad - wbase)
                    mg = msgsp.tile([P, max_chunks * P], BF16, tag="mg")
                    mg3 = mg[:, :nchk * P].rearrange("p (k f) -> p k f", f=P)
                    # the SWDGE ring can't take much over 8K descriptors in
                    # one self-triggered gather; sub-split large calls
                    for a in range(0, L, GMAX):
                        b = min(a + GMAX, L)
                        nc.gpsimd.dma_gather(
                            mg3[:, a // P:b // P, :],
                            hst[wbase:wbase + wsz, :],
                            idx_sb[:, (o + a) // 16:(o + b) // 16],
                            b - a, b - a, P,
                            single_packet=False,
                        )
                    kbase = o // P  # calls are 128-aligned
                    for (k, rt, v) in jobs[(s, wi)]:
                        tl = rt - ts0
                        kk = kbase + k
                        S_t = sonep.tile([P, P], BF16, tag="S")
                        # S = ((iota + 128*v) == rowloc); the ALU runs in
                        # fp32, so offset iota values stay exact
                        nc.vector.tensor_scalar(
                            S_t[:], iot_sb[:],
                            float(v * P), rl_sb[:, kk:kk + 1],
                            mybir.AluOpType.add,
                            mybir.AluOpType.is_equal,
                        )
                        nc.tensor.matmul(
                            pbs[tl][:],
                            S_t[:], mg3[:, k, :],
                            start=(first_job[rt] == (wi, k, rt)),
                            stop=(last_job[rt] == (wi, k, rt)),
                        )
                # evacuate supertile: out = psum * deg_dst + bias
                # (on ACT - keeps DVE free for the one-hot builds)
                ot = osb.tile([P, ST * P], F32, tag="ot")
                for tl in range(nts):
                    tg = ts0 + tl
                    nc.scalar.activation(
                        ot[:, tl * P:(tl + 1) * P],
                        pbs[tl][:],
                        mybir.ActivationFunctionType.Copy,
                        scale=degd_sb[:, tg:tg + 1],
                    )
                    if not bias_zero:
                        nc.vector.tensor_tensor(
                            ot[:, tl * P:(tl + 1) * P],
                            ot[:, tl * P:(tl + 1) * P],
                            bias_sb[:], mybir.AluOpType.add,
                        )
                ov = out[:, :].rearrange("(p j) f -> p j f", p=P)[:, ts0:ts0 + nts, :]
                nc.sync.dma_start(
                    ov, ot[:, :nts * P].rearrange("p (j f) -> p j f", f=P))

    nc.compile()
    return nc


def _pack_shared(x, weight, bias, deg, plan):
    n_local, n_pad, TA = plan["n_local"], plan["n_pad"], plan["n_pad"] // P
    xp = np.zeros((n_pad, P), dtype=np.float32)
    xp[:n_local] = x
    xt = np.ascontiguousarray(xp.T.astype(npbf16))
    wb = weight.astype(npbf16)
    dg = np.zeros((TA * P,), dtype=np.float32)
    dg[:n_local] = deg
    dega = np.ascontiguousarray(dg.reshape(TA, P).T)
    iot = np.ascontiguousarray(
        np.broadcast_to(np.arange(P, dtype=np.float32), (P, P))).astype(npbf16)
    biasb = np.ascontiguousarray(
        np.broadcast_to(bias.astype(np.float32), (P, P)))
    return xt, wb, dega, iot, biasb


def _pack_degd(deg, plan, k):
    own, T, n_local = plan["own"], plan["T"], plan["n_local"]
    d = np.zeros((T * P,), dtype=np.float32)
    n = min(own, n_local - k * own)
    d[:n] = deg[k * own:k * own + n]
    return np.ascontiguousarray(d.reshape(T, P).T)


_CACHE = {}


def _unpack_out(arr, plan):
    # out rows are partition-major (row = p*T + t); undo on host
    T, own = plan["T"], plan["own"]
    return np.asarray(arr).reshape(P, T, P).transpose(1, 0, 2).reshape(T * P, P)[:own]


def kernel(x, weight, bias, deg_inv_sqrt, row, col, num_owned,
           _want_trace=False):
    n_local = int(x.shape[0])
    n_owned = int(num_owned)
    x = np.asarray(x, dtype=np.float32)
    weight = np.asarray(weight, dtype=np.float32)
    bias = np.asarray(bias, dtype=np.float32)
    deg = np.asarray(deg_inv_sqrt, dtype=np.float32)

    plan, per_core = _plan(row, col, n_local, n_owned)
    xt, wb, dega, iot, biasb = _pack_shared(x, weight, bias, deg, plan)

    bias_zero = bool(np.all(bias == 0.0))
    sig = (n_local, n_owned, plan["TOT"], plan["C"].tobytes(), bias_zero)
    if sig in _CACHE:
        nc = _CACHE[sig]
    else:
        nc = _build(plan, bias_zero=bias_zero)
        _CACHE[sig] = nc

    in_maps = []
    for k in range(N_CORES):
        idxk, rlk = per_core[k]
        in_maps.append(dict(
            xt=xt, wgt=wb, dega=dega, iot=iot, biasb=biasb,
            idx=np.ascontiguousarray(idxk), rl=rlk,
            degd=_pack_degd(deg, plan, k),
        ))

    res = run_bass_kernel_spmd(nc, in_maps, core_ids=list(range(N_CORES)),
                               trace=_want_trace)

    own, T = plan["own"], plan["T"]
    full = np.empty((n_owned, P), dtype=np.float32)
    for k in range(N_CORES):
        full[k * own:(k + 1) * own] = _unpack_out(res.results[k]["out"], plan)
    kernel.last_results = res
    return full



# revision 2
# speedup vs baseline: 1.5704x; 1.5704x over previous
"""Distributed GCNConv kernel for Trainium2 (8 NeuronCores).

Graph-partition (expert-style) sharding by destination node: core k owns
destination rows [k*num_owned/8, (k+1)*num_owned/8). Edges whose dst lies
outside every core's owned range (row >= num_owned) are dropped (the
reference discards those aggregates).

Aggregation commutes with the weight matmul ((A.xs).W == A.(xs.W)), so the
device never materializes h = xs @ W. Per core (SPMD, one NEFF):

  the host packs xs = deg_src * x into a bf16 partition-major HBM table
  (row of node n = (n%128)*TA + n//128); edges sorted by (dst supertile,
  32K source window, dst tile); per-(tile,window) slot counts are padded
  only to the max across cores (SPMD uniformity). One gpsimd.dma_gather
  per (supertile, window) (int16 window-relative indices, single gather
  call of up to ~5K descs, single_packet=False) pulls 256B messages; DVE
  builds a one-hot S per 128-edge chunk via tensor_scalar((iota + 128*v)
  == rowloc) where v selects the dst tile for chunks spanning tile
  boundaries; PE accumulates psum[feat, dst] += msgs^T @ S with one PSUM
  bank per dst tile; after a supertile, ACT evacuates aggT to SBUF bf16
  and PE runs out^T = W^T @ aggT per tile (reusing the freed banks); the
  raw out^T[oc, dst] store is unscaled - the host applies deg_dst, bias
  and the transpose.

Self-contained: only needs numpy / ml_dtypes / the concourse Bass stack.
"""

import numpy as np
import ml_dtypes

import concourse.bacc as bacc
import concourse.bass as bass
import concourse.mybir as mybir
import concourse.tile as tile
from concourse.bass_utils import run_bass_kernel_spmd

P = 128
N_CORES = 8
WIN = 32768  # int16 gather-index window (dma_gather idxs are int16)
ST = 8       # dst tiles per supertile (one PSUM bank per tile, 8 banks)
GMAX = 8192  # max gather indices per dma_gather call (SWDGE ring limit)
BF16 = mybir.dt.bfloat16
F32 = mybir.dt.float32
I16 = mybir.dt.int16
npbf16 = ml_dtypes.bfloat16
PAD_ROWLOC = -1.0  # one-hot miss marker (never equals iota + 128*v >= 0)


def _wrap_idx(a):
    """dma_gather index layout: linear index i lives at [i%16, i//16],
    replicated across the 8 Q7 cores -> [128, len//16]."""
    return np.tile(a.reshape(-1, 16).T, (8, 1))


def _plan(row, col, n_local, n_owned):
    """Host-side graph partitioning. Returns the SPMD-uniform schedule and
    the per-core packed index/rowloc arrays."""
    own = n_owned // N_CORES
    T = -(-own // P)                    # dst tiles per core
    S = -(-T // ST)                     # supertiles per core
    NW = -(-n_local // WIN)             # gather windows
    n_pad = -(-n_local // P) * P        # padded node count

    TA = n_pad // P

    row = np.asarray(row).astype(np.int64)
    col = np.asarray(col).astype(np.int64)
    keep = row < n_owned
    r, c = row[keep], col[keep]
    core = r // own
    rl = r - core * own
    t = rl // P
    # the xs table is stored partition-major (table row of node n is
    # (n%128)*TA + n//128); gather indices use table rows
    c = (c % P) * TA + (c // P)
    w = c // WIN

    # slot counts per (core, t, w); pad only to the max across cores (the
    # program must be SPMD-uniform). Runs pack back-to-back UNALIGNED; a
    # 128-edge chunk that spans tile boundaries issues one matmul per
    # covered tile, with the one-hot built against an offset iota slice.
    key = (core * T + t) * NW + w
    counts = np.bincount(key, minlength=N_CORES * T * NW).reshape(N_CORES, T, NW)
    C = counts.max(axis=0).astype(np.int64)  # [T, NW]
    assert counts.sum(axis=(0, 2)).min() > 0, "empty dst tile"

    # run order: supertile-major, then window, then tile (so each (s,w) is
    # one contiguous same-window gather call)
    runs = []   # (s, w, t, slot_off)
    calls = []  # (s, w, slot_off, L, tail) per (s,w), L 128-aligned; the
                # last `tail` slots are common pad (gathered as idx -1,
                # which the DMA skips)
    jobs = {}   # (s, w) -> list of (chunk_local, tile, iota_variant)
    first_job = {}  # tile -> (w, chunk_local, tile)
    last_job = {}
    off = 0
    t0_of_chunk = []  # global chunk -> first covered tile
    for s in range(S):
        ts = range(s * ST, min((s + 1) * ST, T))
        for wi in range(NW):
            call_off = off
            run_list = []
            for ti in ts:
                if C[ti, wi]:
                    runs.append((s, wi, ti, off))
                    run_list.append((ti, off, int(C[ti, wi])))
                    off += int(C[ti, wi])
            off_real = off
            L = -(-(off - call_off) // P) * P
            if not L:
                continue
            off = call_off + L
            calls.append((s, wi, call_off, L, call_off + L - off_real))
            nchk = L // P
            t0 = [None] * nchk
            jlist = []
            for (ti, o_r, cnt) in run_list:
                k_first = (o_r - call_off) // P
                k_last = (o_r + cnt - 1 - call_off) // P
                for k in range(k_first, k_last + 1):
                    if t0[k] is None:
                        t0[k] = ti
                    jlist.append((k, ti))
            jlist.sort()
            jlist2 = []
            for (k, ti) in jlist:
                v = ti - t0[k]
                assert 0 <= v < 64, (ti, t0[k])
                jlist2.append((k, ti, v))
                jk = (wi, k, ti)
                if ti not in first_job:
                    first_job[ti] = jk
                last_job[ti] = jk
            jobs[(s, wi)] = jlist2
            t0_of_chunk.extend(ti if ti is not None else 0 for ti in t0)
    TOT = off
    NCHK = TOT // P
    T0g = np.asarray(t0_of_chunk, dtype=np.int64)
    assert len(T0g) == NCHK

    # per-core slot packing
    per_core = []
    for k in range(N_CORES):
        m = core == k
        ck, rlk, tk, wk = c[m], rl[m], t[m], w[m]
        # composite key in stream order: (supertile, window, tile)
        g = (tk // ST) * (NW * T) + wk * T + tk
        order = np.argsort(g, kind="stable")
        ck, rlk, tk, wk, g = ck[order], rlk[order], tk[order], wk[order], g[order]
        ngrp = S * NW * T
        cnt = np.bincount(g, minlength=ngrp)
        starts = np.concatenate(([0], np.cumsum(cnt)))[:-1]
        rank = np.arange(len(g)) - starts[g]
        # map stream-order key -> run slot offset
        lut = np.full(ngrp, -1, dtype=np.int64)
        for (si, wi, ti, o) in runs:
            lut[si * (NW * T) + wi * T + ti] = o
        dest = lut[g] + rank
        assert dest.min() >= 0

        gidx = np.zeros(TOT, dtype=np.int16)
        rloc = np.full(TOT, PAD_ROWLOC, dtype=np.float32)
        gidx[dest] = (ck - wk * WIN).astype(np.int16)
        # rowloc relative to the chunk's FIRST covered tile: tiles later in
        # the chunk land at +128/+256 (matched by the offset iota slices)
        rloc[dest] = (rlk - tk * P + P * (tk - T0g[dest // P])).astype(np.float32)

        idx_parts = []
        for (_, _, o, L, _) in calls:
            for a in range(0, L, GMAX):
                b = min(a + GMAX, L)
                idx_parts.append(_wrap_idx(gidx[o + a:o + b]))
        idx_wrapped = np.concatenate(idx_parts, axis=1)
        RL = np.ascontiguousarray(rloc.reshape(NCHK, P).T)
        per_core.append((idx_wrapped, RL))

    plan = dict(
        n_local=n_local, n_owned=n_owned, own=own, T=T, S=S, NW=NW,
        n_pad=n_pad, C=C, runs=runs, calls=calls, jobs=jobs, TOT=TOT,
        NCHK=NCHK, first_job=first_job, last_job=last_job,
    )
    return plan, per_core


def _build(plan):
    """Emit the Bass/Tile program (identical for all cores)."""
    n_pad, T, S, NW = plan["n_pad"], plan["T"], plan["S"], plan["NW"]
    C, calls, jobs = plan["C"], plan["calls"], plan["jobs"]
    TOT, NCHK = plan["TOT"], plan["NCHK"]
    first_job, last_job = plan["first_job"], plan["last_job"]

    nc = bacc.Bacc("TRN2", target_bir_lowering=False, debug=False,
                   enable_asserts=False, num_devices=N_CORES)

    xst = nc.dram_tensor("xst", [n_pad, P], BF16, kind="ExternalInput")
    wgt = nc.dram_tensor("wgt", [P, P], BF16, kind="ExternalInput")
    iot = nc.dram_tensor("iot", [P, P], BF16, kind="ExternalInput")
    idx = nc.dram_tensor("idx", [P, TOT // 16], I16, kind="ExternalInput")
    rld = nc.dram_tensor("rl", [P, NCHK], F32, kind="ExternalInput")
    out = nc.dram_tensor("out", [P, T * P], BF16, kind="ExternalOutput")

    with tile.TileContext(nc) as tc:
        with (
            tc.tile_pool(name="const", bufs=1) as constp,
            # one shared PSUM pool: every tile is one full bank (the HW
            # zeroes accumulation state at bank granularity, so each
            # accumulator group must own its bank); the per-tile out
            # matmuls reuse the same 8 banks round-robin after evacuation
            tc.tile_pool(name="ps", bufs=8, space="PSUM") as psp,
            tc.tile_pool(name="msgs", bufs=4) as msgsp,
            tc.tile_pool(name="sone", bufs=12) as sonep,
            tc.tile_pool(name="agg", bufs=2) as aggp,
            tc.tile_pool(name="osb", bufs=2) as osb,
        ):
            # resident constants / metadata
            w_sb = constp.tile([P, P], BF16)
            nc.sync.dma_start(w_sb[:], wgt[:, :])
            iot_sb = constp.tile([P, P], BF16)
            nc.sync.dma_start(iot_sb[:], iot[:, :])
            idx_sb = constp.tile([P, TOT // 16], I16)
            nc.sync.dma_start(idx_sb[:], idx[:, :])
            rl_sb = constp.tile([P, NCHK], F32)
            nc.sync.dma_start(rl_sb[:], rld[:, :])

            max_chunks = max(L for (_, _, _, L, _) in calls) // P

            for s in range(S):
                ts0 = s * ST
                nts = min(ST, T - ts0)
                # one PSUM bank per dst tile; the group stays open across
                # all gather windows of this supertile. partition dim is
                # the feature: psum[feat, dst] += msgs^T @ S
                pbs = [psp.tile([P, P], F32, tag="pb", name=f"pb{s}_{i}")
                       for i in range(nts)]
                for (cs, wi, o, L, tail) in calls:
                    if cs != s:
                        continue
                    nchk = L // P
                    wbase = wi * WIN
                    wsz = min(WIN, n_pad - wbase)
                    mg = msgsp.tile([P, max_chunks * P], BF16, tag="mg")
                    mg3 = mg[:, :nchk * P].rearrange("p (k f) -> p k f", f=P)
                    for a in range(0, L, GMAX):
                        b = min(a + GMAX, L)
                        nc.gpsimd.dma_gather(
                            mg3[:, a // P:b // P, :],
                            xst[wbase:wbase + wsz, :],
                            idx_sb[:, (o + a) // 16:(o + b) // 16],
                            b - a, b - a, P,
                            single_packet=False,
                        )
                    kbase = o // P  # calls are 128-aligned
                    for (k, rt, v) in jobs[(s, wi)]:
                        tl = rt - ts0
                        kk = kbase + k
                        S_t = sonep.tile([P, P], BF16, tag="S")
                        # S = ((iota + 128*v) == rowloc); the ALU runs in
                        # fp32, so offset iota values stay exact
                        nc.vector.tensor_scalar(
                            S_t[:], iot_sb[:],
                            float(v * P), rl_sb[:, kk:kk + 1],
                            mybir.AluOpType.add,
                            mybir.AluOpType.is_equal,
                        )
                        nc.tensor.matmul(
                            pbs[tl][:],
                            mg3[:, k, :], S_t[:],
                            start=(first_job[rt] == (wi, k, rt)),
                            stop=(last_job[rt] == (wi, k, rt)),
                        )
                # evacuate supertile aggT to SBUF bf16 (on ACT - keeps DVE
                # free for the one-hot builds), then out^T = W^T @ aggT per
                # tile, reusing the freed banks
                ag = aggp.tile([P, ST * P], BF16, tag="ag")
                for tl in range(nts):
                    nc.scalar.activation(
                        ag[:, tl * P:(tl + 1) * P],
                        pbs[tl][:],
                        mybir.ActivationFunctionType.Copy,
                    )
                ot = osb.tile([P, ST * P], BF16, tag="ot")
                for tl in range(nts):
                    po = psp.tile([P, P], F32, tag="pb", name=f"po{s}_{tl}")
                    nc.tensor.matmul(
                        po[:], w_sb[:], ag[:, tl * P:(tl + 1) * P],
                        start=True, stop=True,
                    )
                    nc.scalar.activation(
                        ot[:, tl * P:(tl + 1) * P],
                        po[:],
                        mybir.ActivationFunctionType.Copy,
                    )
                nc.sync.dma_start(
                    out[:, ts0 * P:(ts0 + nts) * P], ot[:, :nts * P])

    nc.compile()
    return nc


def _pack_xs(x, deg, plan):
    """deg_src-scaled x in the partition-major table layout (row of node n
    is (n%128)*TA + n//128), bf16."""
    n_local, n_pad = plan["n_local"], plan["n_pad"]
    TA = n_pad // P
    xp = np.zeros((n_pad, P), dtype=np.float32)
    xp[:n_local] = deg[:, None] * x
    xst = xp.reshape(TA, P, P).transpose(1, 0, 2).reshape(n_pad, P)
    return np.ascontiguousarray(xst).astype(npbf16)


_CACHE = {}


def kernel(x, weight, bias, deg_inv_sqrt, row, col, num_owned,
           _want_trace=False):
    n_local = int(x.shape[0])
    n_owned = int(num_owned)
    x = np.asarray(x, dtype=np.float32)
    weight = np.asarray(weight, dtype=np.float32)
    bias = np.asarray(bias, dtype=np.float32)
    deg = np.asarray(deg_inv_sqrt, dtype=np.float32)

    plan, per_core = _plan(row, col, n_local, n_owned)
    xst = _pack_xs(x, deg, plan)
    wb = weight.astype(npbf16)
    iot = np.ascontiguousarray(
        np.broadcast_to(np.arange(P, dtype=np.float32), (P, P))).astype(npbf16)

    sig = (n_local, n_owned, plan["TOT"], plan["C"].tobytes())
    if sig in _CACHE:
        nc = _CACHE[sig]
    else:
        nc = _build(plan)
        _CACHE[sig] = nc

    in_maps = []
    for k in range(N_CORES):
        idxk, rlk = per_core[k]
        in_maps.append(dict(
            xst=xst, wgt=wb, iot=iot,
            idx=np.ascontiguousarray(idxk), rl=rlk,
        ))

    res = run_bass_kernel_spmd(nc, in_maps, core_ids=list(range(N_CORES)),
                               trace=_want_trace)

    own, T = plan["own"], plan["T"]
    full = np.empty((n_owned, P), dtype=np.float32)
    for k in range(N_CORES):
        outT = np.asarray(res.results[k]["out"], dtype=np.float32)  # [P, T*P]
        full[k * own:(k + 1) * own] = outT.T[:own]
    full *= deg[:n_owned, None]
    full += bias
    kernel.last_results = res
    return full


# revision 12
# speedup vs baseline: 2.5213x; 1.6055x over previous
"""Distributed GCNConv kernel for Trainium2 (8 NeuronCores).

Graph-partition (expert-style) sharding by destination node: core k owns
destination rows [k*num_owned/8, (k+1)*num_owned/8). Edges whose dst lies
outside every core's owned range (row >= num_owned) are dropped (the
reference discards those aggregates).

Aggregation commutes with the weight matmul ((A.xs).W == A.(xs.W)), so the
device never materializes h = xs @ W; it gathers deg_src-scaled x messages
straight from a host-packed table and aggregates into PSUM.

Paired gather: dma_gather descriptors under 512B pay a 2x DMA penalty, so
the host packs TWO nodes per 512B table row and pairs up edges whose dsts
share a tile (each node may join at most 2 rows - bounded halo
replication). A paired descriptor serves two edges for the price of one;
~40% of descriptors are eliminated. Rows are assigned round-robin to two
32768-row windows (int16 gather indices).

Per core (SPMD, one NEFF): edges sorted by (dst supertile, window, dst
tile); per-(tile,window) run the slots are laid out [half1-singles][pairs]
[half0-singles] with each section padded to the max across cores (SPMD
uniformity), so the job schedule is core-independent. One gpsimd.dma_gather
per (supertile, window) pulls 512B rows; DVE builds a one-hot S per
128-slot chunk and half via tensor_scalar((iota + 128*v) == rowloc); PE
accumulates psum[feat, dst] += msgs_half^T @ S with one PSUM bank per dst
tile; after a supertile, ACT evacuates aggT to SBUF bf16 and PE runs
out^T = W^T @ aggT per tile (reusing the freed banks); the raw
out^T[oc, dst] store is unscaled - the host applies deg_dst, bias and the
transpose.

Self-contained: only needs numpy / ml_dtypes / the concourse Bass stack.
"""

import numpy as np
import ml_dtypes

import concourse.bacc as bacc
import concourse.bass as bass
import concourse.mybir as mybir
import concourse.tile as tile
from concourse.bass_utils import run_bass_kernel_spmd

P = 128
N_CORES = 8
WINROWS = 32768  # rows per gather window (int16 indices)
NW = 3           # windows (table = 98304 rows of 512B)
ROWS = NW * WINROWS
ST = 8           # dst tiles per supertile (one PSUM bank per tile, 8 banks)
GMAX = 2048      # max gather indices per dma_gather call (pipelining grain)
BUD = 2          # max table rows a node may join (halo replication bound)
BF16 = mybir.dt.bfloat16
F32 = mybir.dt.float32
I16 = mybir.dt.int16
npbf16 = ml_dtypes.bfloat16
PAD_ROWLOC = -1.0  # one-hot miss marker (never equals iota + 128*v >= 0)


def _wrap_idx(a):
    """dma_gather index layout: linear index i lives at [i%16, i//16],
    replicated across the 8 Q7 cores -> [128, len//16]."""
    return np.tile(a.reshape(-1, 16).T, (8, 1))


def _pair_core(nk, tk, rlk, n_local, T):
    """Budget-2 greedy pairing of this core's edges within each dst tile.

    Returns per-slot arrays (slot entities):
      s_t     [M] tile of slot
      s_row   [M] table row id (core-local, before window assignment)
      s_sec   [M] section: 0 = half1-single, 1 = pair, 2 = half0-single
      s_rl0   [M] dst local row of the half-0 edge (or -1)
      s_rl1   [M] dst local row of the half-1 edge (or -1)
    plus rows_node [R, 2] (node of each half; n_local = zero pad) and
    rows_tile [R] (tile used for window balancing).
    """
    E = len(nk)
    budget = np.full(n_local, BUD, dtype=np.int8)
    # node -> first assigned (row, half) for uncovered edges
    node_row = np.full(n_local, -1, dtype=np.int64)
    node_half = np.zeros(n_local, dtype=np.int8)

    order = np.argsort(tk, kind="stable")
    ns, ts, rs = nk[order], tk[order], rlk[order]
    bounds = np.searchsorted(ts, np.arange(T + 1))

    rows_a, rows_b, rows_t = [], [], []
    s_t, s_row, s_sec, s_rl0, s_rl1 = [], [], [], [], []
    single_e = []  # (node, tile, rloc) uncovered edges

    for ti in range(T):
        lo, hi = bounds[ti], bounds[ti + 1]
        seg_n, seg_r = ns[lo:hi], rs[lo:hi]
        av = budget[seg_n] > 0
        idx_av = np.flatnonzero(av)
        npair = len(idx_av) // 2
        for i in range(npair):
            ia, ib = idx_av[2 * i], idx_av[2 * i + 1]
            na, nb = int(seg_n[ia]), int(seg_n[ib])
            r = len(rows_a)
            rows_a.append(na)
            rows_b.append(nb)
            rows_t.append(ti)
            budget[na] -= 1
            budget[nb] -= 1
            if node_row[na] < 0:
                node_row[na] = r
                node_half[na] = 0
            if node_row[nb] < 0:
                node_row[nb] = r
                node_half[nb] = 1
            s_t.append(ti)
            s_row.append(r)
            s_sec.append(1)
            s_rl0.append(int(seg_r[ia]))
            s_rl1.append(int(seg_r[ib]))
        for i in idx_av[2 * npair:]:
            single_e.append((int(seg_n[i]), ti, int(seg_r[i])))
        for i in np.flatnonzero(~av):
            single_e.append((int(seg_n[i]), ti, int(seg_r[i])))

    # second pass: pair the leftover edges with each other within a tile
    # (each node gets at most ONE extra row membership -> <=3 appearances)
    bud2 = np.ones(n_local, dtype=np.int8)
    single2 = []
    by_tile = {}
    for e in single_e:
        by_tile.setdefault(e[1], []).append(e)
    for ti in sorted(by_tile):
        pend = None
        for (n, _, rr) in by_tile[ti]:
            if not bud2[n]:
                single2.append((n, ti, rr))
                continue
            if pend is None:
                pend = (n, rr)
                continue
            na, ra = pend
            pend = None
            r = len(rows_a)
            rows_a.append(na)
            rows_b.append(n)
            rows_t.append(ti)
            bud2[na] = 0
            bud2[n] = 0
            if node_row[na] < 0:
                node_row[na] = r
                node_half[na] = 0
            if node_row[n] < 0:
                node_row[n] = r
                node_half[n] = 1
            s_t.append(ti)
            s_row.append(r)
            s_sec.append(1)
            s_rl0.append(ra)
            s_rl1.append(rr)
        if pend is not None:
            single2.append((pend[0], ti, pend[1]))
    single_e = single2

    # leftover nodes (edges but never placed in a row): pair arbitrarily
    pend = None
    for (n, ti, _) in single_e:
        if node_row[n] >= 0:
            continue
        if pend is None:
            pend = n
            r = len(rows_a)
            rows_a.append(n)
            rows_b.append(n_local)  # zero pad node
            rows_t.append(ti)
            node_row[n] = r
            node_half[n] = 0
        elif pend != n:
            rows_b[node_row[pend]] = n
            node_row[n] = node_row[pend]
            node_half[n] = 1
            pend = None

    for (n, ti, rr) in single_e:
        r = int(node_row[n])
        h = int(node_half[n])
        s_t.append(ti)
        s_row.append(r)
        s_sec.append(0 if h == 1 else 2)
        s_rl0.append(rr if h == 0 else -1)
        s_rl1.append(rr if h == 1 else -1)

    rows_node = np.stack(
        [np.asarray(rows_a, dtype=np.int64), np.asarray(rows_b, dtype=np.int64)],
        axis=1,
    )
    return (np.asarray(s_t, dtype=np.int64), np.asarray(s_row, dtype=np.int64),
            np.asarray(s_sec, dtype=np.int64), np.asarray(s_rl0, dtype=np.int64),
            np.asarray(s_rl1, dtype=np.int64), rows_node,
            np.asarray(rows_t, dtype=np.int64))


def _plan(row, col, n_local, n_owned):
    """Host-side graph partitioning + pairing. Returns the SPMD-uniform
    schedule and the per-core packed tables."""
    own = n_owned // N_CORES
    T = -(-own // P)                    # dst tiles per core
    S = -(-T // ST)                     # supertiles per core

    row = np.asarray(row).astype(np.int64)
    col = np.asarray(col).astype(np.int64)
    keep = row < n_owned
    r, c = row[keep], col[keep]
    core_of = r // own
    rl_of = r - core_of * own

    per_core_raw = []
    cnt = np.zeros((N_CORES, T, NW, 3), dtype=np.int64)
    for k in range(N_CORES):
        m = core_of == k
        nk, rlk = c[m], rl_of[m]
        tk = rlk // P
        s_t, s_row, s_sec, s_rl0, s_rl1, rows_node, rows_t = _pair_core(
            nk, tk, rlk, n_local, T)
        # window assignment: per tile, alternate hit rows between windows;
        # then round-robin the leftovers. bucket0 -> rows 0.., bucket1 ->
        # rows 32768..
        R = len(rows_node)
        order = np.argsort(rows_t, kind="stable")
        alt = np.zeros(R, dtype=np.int64)
        # alternate within each tile group
        tt = rows_t[order]
        first = np.concatenate(([True], tt[1:] != tt[:-1]))
        grp_start = np.flatnonzero(first)
        pos = np.arange(R) - np.repeat(grp_start, np.diff(
            np.concatenate((grp_start, [R]))))
        alt[order] = pos % NW
        row_gidx = np.empty(R, dtype=np.int64)
        for wbkt in range(NW):
            bw = np.flatnonzero(alt == wbkt)
            assert len(bw) <= WINROWS, (wbkt, len(bw))
            row_gidx[bw] = wbkt * WINROWS + np.arange(len(bw))
        s_g = row_gidx[s_row]
        s_w = s_g // WINROWS
        np.add.at(cnt, (k, s_t, s_w, s_sec), 1)
        per_core_raw.append((s_t, s_w, s_g - s_w * WINROWS, s_sec,
                             s_rl0, s_rl1, rows_node, row_gidx))

    C3 = cnt.max(axis=0)  # [T, NW, 3] per-section maxima
    assert C3.sum(axis=(1, 2)).min() > 0, "empty dst tile"

    # geometry: supertile-major, then window, then tile; per run the three
    # sections pack back-to-back [h1-singles][pairs][h0-singles]. Runs pack
    # UNALIGNED; chunks spanning tile boundaries issue one matmul per
    # covered (tile, half) with the one-hot built against an offset iota.
    runs = []   # (s, w, t, slot_off)  (slot_off = run start)
    calls = []  # (s, w, slot_off, L) per (s,w), L 128-aligned
    jobs = {}   # (s, w) -> list of (chunk_local, tile, iota_variant, half)
    first_job = {}
    last_job = {}
    sec_base = {}  # (t, w) -> run start offset
    off = 0
    t0_of_chunk = []
    for s in range(S):
        ts = range(s * ST, min((s + 1) * ST, T))
        for wi in range(NW):
            call_off = off
            run_list = []
            for ti in ts:
                n1, nh, n0 = (int(C3[ti, wi, 0]), int(C3[ti, wi, 1]),
                              int(C3[ti, wi, 2]))
                ln = n1 + nh + n0
                if ln:
                    runs.append((s, wi, ti, off))
                    sec_base[(ti, wi)] = off
                    run_list.append((ti, off, n1, nh, n0))
                    off += ln
            off_real = off
            L = -(-(off - call_off) // P) * P
            if not L:
                continue
            off = call_off + L
            calls.append((s, wi, call_off, L))
            nchk = L // P
            t0 = [None] * nchk
            for (ti, o_r, n1, nh, n0) in run_list:
                k_first = (o_r - call_off) // P
                k_last = (o_r + n1 + nh + n0 - 1 - call_off) // P
                for k in range(k_first, k_last + 1):
                    if t0[k] is None:
                        t0[k] = ti
            jlist = []
            for (ti, o_r, n1, nh, n0) in run_list:
                for (h, lo, hi) in ((1, o_r, o_r + n1 + nh),
                                    (0, o_r + n1, o_r + n1 + nh + n0)):
                    if hi <= lo:
                        continue
                    k_first = (lo - call_off) // P
                    k_last = (hi - 1 - call_off) // P
                    for k in range(k_first, k_last + 1):
                        v = ti - t0[k]
                        assert 0 <= v < 64
                        jlist.append((k, ti, v, h))
            jlist.sort()
            for (k, ti, v, h) in jlist:
                jk = (wi, k, ti, h)
                if ti not in first_job:
                    first_job[ti] = jk
                last_job[ti] = jk
            jobs[(s, wi)] = jlist
            t0_of_chunk.extend(ti if ti is not None else 0 for ti in t0)
    TOT = off
    NCHK = TOT // P
    T0g = np.asarray(t0_of_chunk, dtype=np.int64)
    assert len(T0g) == NCHK

    # per-core slot packing into the uniform geometry
    per_core = []
    for k in range(N_CORES):
        (s_t, s_w, s_gi, s_sec, s_rl0, s_rl1, rows_node, row_gidx) = \
            per_core_raw[k]
        M = len(s_t)
        # dest slot: section base + rank within (t, w, sec)
        key = (s_t * NW + s_w) * 3 + s_sec
        order = np.argsort(key, kind="stable")
        ks = key[order]
        cnts = np.bincount(key, minlength=T * NW * 3)
        starts = np.concatenate(([0], np.cumsum(cnts)))[:-1]
        rank = np.empty(M, dtype=np.int64)
        rank[order] = np.arange(M) - starts[ks]
        # base of each (t, w, sec) in the global slot space
        base = np.full(T * NW * 3, -1, dtype=np.int64)
        for (ti, wi) in sec_base:
            o_r = sec_base[(ti, wi)]
            n1, nh = int(C3[ti, wi, 0]), int(C3[ti, wi, 1])
            base[(ti * NW + wi) * 3 + 0] = o_r
            base[(ti * NW + wi) * 3 + 1] = o_r + n1
            base[(ti * NW + wi) * 3 + 2] = o_r + n1 + nh
        dest = base[key] + rank
        assert dest.min() >= 0 and dest.max() < TOT

        gidx = np.zeros(TOT, dtype=np.int16)
        rl0 = np.full(TOT, PAD_ROWLOC, dtype=np.float32)
        rl1 = np.full(TOT, PAD_ROWLOC, dtype=np.float32)
        gidx[dest] = s_gi.astype(np.int16)
        # rowloc relative to the chunk's FIRST covered tile
        adj = -s_t * P + P * (s_t - T0g[dest // P])
        rl0[dest] = np.where(s_rl0 >= 0, s_rl0 + adj, PAD_ROWLOC)
        rl1[dest] = np.where(s_rl1 >= 0, s_rl1 + adj, PAD_ROWLOC)

        idx_parts = []
        for (_, _, o, L) in calls:
            for a in range(0, L, GMAX):
                b = min(a + GMAX, L)
                idx_parts.append(_wrap_idx(gidx[o + a:o + b]))
        idx_wrapped = np.concatenate(idx_parts, axis=1)
        # rl table: column 2k = half0 of chunk k, 2k+1 = half1
        RL = np.empty((P, 2 * NCHK), dtype=np.float32)
        RL[:, 0::2] = rl0.reshape(NCHK, P).T
        RL[:, 1::2] = rl1.reshape(NCHK, P).T
        per_core.append((idx_wrapped, np.ascontiguousarray(RL),
                         rows_node, row_gidx))

    plan = dict(
        n_local=n_local, n_owned=n_owned, own=own, T=T, S=S,
        C3=C3, runs=runs, calls=calls, jobs=jobs, TOT=TOT,
        NCHK=NCHK, first_job=first_job, last_job=last_job,
    )
    return plan, per_core


def _build(plan):
    """Emit the Bass/Tile program (identical for all cores)."""
    T, S = plan["T"], plan["S"]
    calls, jobs = plan["calls"], plan["jobs"]
    TOT, NCHK = plan["TOT"], plan["NCHK"]
    first_job, last_job = plan["first_job"], plan["last_job"]

    nc = bacc.Bacc("TRN2", target_bir_lowering=False, debug=False,
                   enable_asserts=False, num_devices=N_CORES)

    xst = nc.dram_tensor("xst", [ROWS, 2 * P], BF16, kind="ExternalInput")
    wgt = nc.dram_tensor("wgt", [P, P], BF16, kind="ExternalInput")
    iot = nc.dram_tensor("iot", [P, P], BF16, kind="ExternalInput")
    idx = nc.dram_tensor("idx", [P, TOT // 16], I16, kind="ExternalInput")
    rld = nc.dram_tensor("rl", [P, 2 * NCHK], F32, kind="ExternalInput")
    out = nc.dram_tensor("out", [P, T * P], BF16, kind="ExternalOutput")

    with tile.TileContext(nc) as tc:
        with (
            tc.tile_pool(name="const", bufs=1) as constp,
            # one shared PSUM pool: every tile is one full bank (the HW
            # zeroes accumulation state at bank granularity, so each
            # accumulator group must own its bank); the per-tile out
            # matmuls reuse the same 8 banks round-robin after evacuation
            tc.tile_pool(name="ps", bufs=8, space="PSUM") as psp,
            tc.tile_pool(name="msgs", bufs=4) as msgsp,
            tc.tile_pool(name="sone", bufs=2) as sonep,
            tc.tile_pool(name="agg", bufs=2) as aggp,
            tc.tile_pool(name="osb", bufs=2) as osb,
        ):
            w_sb = constp.tile([P, P], BF16)
            nc.sync.dma_start(w_sb[:], wgt[:, :])
            iot_sb = constp.tile([P, P], BF16)
            nc.sync.dma_start(iot_sb[:], iot[:, :])
            # idx/rl loaded in per-supertile slices so the first gather's
            # descriptor generation isn't blocked on the full metadata load
            idx_sb = constp.tile([P, TOT // 16], I16)
            rl_sb = constp.tile([P, 2 * NCHK], F32)
            s_lo = {}
            s_hi = {}
            for (cs, wi, o, L) in calls:
                s_lo[cs] = min(s_lo.get(cs, o), o)
                s_hi[cs] = max(s_hi.get(cs, o + L), o + L)
            for s in range(S):
                lo, hi = s_lo[s], s_hi[s]
                nc.sync.dma_start(idx_sb[:, lo // 16:hi // 16],
                                  idx[:, lo // 16:hi // 16])
                nc.sync.dma_start(rl_sb[:, 2 * (lo // P):2 * (hi // P)],
                                  rld[:, 2 * (lo // P):2 * (hi // P)])

            max_chunks = max(L for (_, _, _, L) in calls) // P
            max_jobs = max(len(j) for j in jobs.values())

            for s in range(S):
                ts0 = s * ST
                nts = min(ST, T - ts0)
                # one PSUM bank per dst tile: psum[feat, dst] += msgs^T @ S
                pbs = [psp.tile([P, P], F32, tag="pb", name=f"pb{s}_{i}")
                       for i in range(nts)]
                for (cs, wi, o, L) in calls:
                    if cs != s:
                        continue
                    nchk = L // P
                    wbase = wi * WINROWS
                    mg = msgsp.tile([P, max_chunks * 2 * P], BF16, tag="mg")
                    mg3 = mg[:, :nchk * 2 * P].rearrange(
                        "p (k f) -> p k f", f=2 * P)
                    for a in range(0, L, GMAX):
                        b = min(a + GMAX, L)
                        nc.gpsimd.dma_gather(
                            mg3[:, a // P:b // P, :],
                            xst[wbase:wbase + WINROWS, :],
                            idx_sb[:, (o + a) // 16:(o + b) // 16],
                            b - a, b - a, 2 * P,
                            single_packet=False,
                        )
                    kbase = o // P
                    jl = jobs[(s, wi)]
                    S_w = sonep.tile([P, max_jobs * P], BF16, tag="S")
                    for jj, (k, rt, v, h) in enumerate(jl):
                        tl = rt - ts0
                        kk = kbase + k
                        S_t = S_w[:, jj * P:(jj + 1) * P]
                        # S = ((iota + 128*v) == rowloc); fp32 ALU keeps
                        # offset iota values exact
                        nc.vector.tensor_scalar(
                            S_t, iot_sb[:],
                            float(v * P), rl_sb[:, 2 * kk + h:2 * kk + h + 1],
                            mybir.AluOpType.add,
                            mybir.AluOpType.is_equal,
                        )
                        nc.tensor.matmul(
                            pbs[tl][:],
                            mg3[:, k, h * P:(h + 1) * P], S_t,
                            start=(first_job[rt] == (wi, k, rt, h)),
                            stop=(last_job[rt] == (wi, k, rt, h)),
                        )
                # evacuate supertile aggT to SBUF bf16 (on ACT), then
                # out^T = W^T @ aggT per tile, reusing the freed banks
                ag = aggp.tile([P, ST * P], BF16, tag="ag")
                for tl in range(nts):
                    nc.scalar.activation(
                        ag[:, tl * P:(tl + 1) * P],
                        pbs[tl][:],
                        mybir.ActivationFunctionType.Copy,
                    )
                ot = osb.tile([P, ST * P], BF16, tag="ot")
                for tl in range(nts):
                    po = psp.tile([P, P], F32, tag="pb", name=f"po{s}_{tl}")
                    nc.tensor.matmul(
                        po[:], w_sb[:], ag[:, tl * P:(tl + 1) * P],
                        start=True, stop=True,
                    )
                    nc.scalar.activation(
                        ot[:, tl * P:(tl + 1) * P],
                        po[:],
                        mybir.ActivationFunctionType.Copy,
                    )
                nc.sync.dma_start(
                    out[:, ts0 * P:(ts0 + nts) * P], ot[:, :nts * P])

    nc.compile()
    return nc


def _pack_xst(x, deg, rows_node, row_gidx, n_local):
    """Paired node table: row r = [xs[a] | xs[b]] bf16, placed at its
    assigned window position."""
    xs = np.zeros((n_local + 1, P), dtype=np.float32)
    xs[:n_local] = deg[:, None] * x
    tbl = np.zeros((ROWS, 2 * P), dtype=npbf16)
    xs16 = xs.astype(npbf16)
    tbl[row_gidx, :P] = xs16[rows_node[:, 0]]
    tbl[row_gidx, P:] = xs16[rows_node[:, 1]]
    return np.ascontiguousarray(tbl)


_CACHE = {}


def kernel(x, weight, bias, deg_inv_sqrt, row, col, num_owned,
           _want_trace=False):
    n_local = int(x.shape[0])
    n_owned = int(num_owned)
    x = np.asarray(x, dtype=np.float32)
    weight = np.asarray(weight, dtype=np.float32)
    bias = np.asarray(bias, dtype=np.float32)
    deg = np.asarray(deg_inv_sqrt, dtype=np.float32)

    plan, per_core = _plan(row, col, n_local, n_owned)
    wb = weight.astype(npbf16)
    iot = np.ascontiguousarray(
        np.broadcast_to(np.arange(P, dtype=np.float32), (P, P))).astype(npbf16)

    sig = (n_local, n_owned, plan["TOT"], plan["C3"].tobytes())
    if sig in _CACHE:
        nc = _CACHE[sig]
    else:
        nc = _build(plan)
        _CACHE[sig] = nc

    in_maps = []
    for k in range(N_CORES):
        idxk, rlk, rows_node, row_gidx = per_core[k]
        in_maps.append(dict(
            xst=_pack_xst(x, deg, rows_node, row_gidx, n_local),
            wgt=wb, iot=iot,
            idx=np.ascontiguousarray(idxk), rl=rlk,
        ))

    res = run_bass_kernel_spmd(nc, in_maps, core_ids=list(range(N_CORES)),
                               trace=_want_trace)

    own, T = plan["own"], plan["T"]
    full = np.empty((n_owned, P), dtype=np.float32)
    for k in range(N_CORES):
        outT = np.asarray(res.results[k]["out"], dtype=np.float32)  # [P, T*P]
        full[k * own:(k + 1) * own] = outT.T[:own]
    full *= deg[:n_owned, None]
    full += bias
    kernel.last_results = res
    return full


# revision 17
# speedup vs baseline: 2.8485x; 1.1298x over previous
"""Distributed GCNConv kernel for Trainium2 (8 NeuronCores).

Graph-partition (expert-style) sharding by destination node: core k owns
destination rows [k*num_owned/8, (k+1)*num_owned/8). Edges whose dst lies
outside every core's owned range (row >= num_owned) are dropped (the
reference discards those aggregates).

Aggregation commutes with the weight matmul ((A.xs).W == A.(xs.W)), so the
device never materializes h = xs @ W; it gathers deg_src-scaled x messages
straight from a host-packed table and aggregates into PSUM.

Paired gather: dma_gather descriptors under 512B pay a 2x DMA penalty, so
the host packs TWO nodes per 512B table row and pairs up edges that share a
DESTINATION (bounded halo replication: a node joins at most ~4 rows, 2x on
average). A paired descriptor serves two edges for the price of one, and
because both halves share the destination, ONE one-hot column drives a
single N=256 matmul that scatters both messages. Uncovered edges ride
[x_n | 0] rows, so every slot is uniform. Rows are dealt round-robin into
three 32768-row windows (int16 gather indices), which also balances the
per-(tile,window) slot counts across cores (SPMD padding stays small).

Per core (SPMD, one NEFF): slots sorted by (dst supertile, window, dst
tile) with per-(tile,window) counts padded to the max across cores. One
gpsimd.dma_gather per (supertile, window) pulls 512B rows (2048-index
sub-calls for pipelining); DVE builds a one-hot S per 128-slot chunk via
tensor_scalar((iota + 128*v) == rowloc) into a per-call wide S tile (one
WAR wait per call instead of per chunk); PE accumulates
psum[dst, 0:256] += S^T @ msg-pairs with one PSUM bank per dst tile. After
a supertile: DVE folds the halves (agg = psum[:,0:128] + psum[:,128:256])
to bf16, PE transposes each tile (identity matmul), ACT evacuates, PE runs
out^T = W^T @ aggT (N=512), ACT evacuates, and the raw out^T[oc, dst]
store is unscaled - the host applies deg_dst, bias and the transpose.

Self-contained: only needs numpy / ml_dtypes / the concourse Bass stack.
"""

import numpy as np
import ml_dtypes

import concourse.bacc as bacc
import concourse.bass as bass
import concourse.mybir as mybir
import concourse.tile as tile
from concourse.bass_utils import run_bass_kernel_spmd

P = 128
N_CORES = 8
WINROWS = 32768  # rows per gather window (int16 indices)
NW = 3           # windows (table = 98304 rows of 512B)
ROWS = NW * WINROWS
ST = 8           # dst tiles per supertile (one PSUM bank per tile, 8 banks)
GMAX = 2048      # max gather indices per dma_gather call (pipelining grain)
BUD = 2          # pass-1 pairing budget per node (pass 2 adds one more)
BF16 = mybir.dt.bfloat16
F32 = mybir.dt.float32
I16 = mybir.dt.int16
npbf16 = ml_dtypes.bfloat16
PAD_ROWLOC = -1.0  # one-hot miss marker (never equals iota + 128*v >= 0)


def _wrap_idx(a):
    """dma_gather index layout: linear index i lives at [i%16, i//16],
    replicated across the 8 Q7 cores -> [128, len//16]."""
    return np.tile(a.reshape(-1, 16).T, (8, 1))


def _pair_core(nk, rlk, n_local):
    """Greedy same-destination pairing of this core's edges.

    Pass 1 pairs edges of the same dst while both nodes have budget (BUD);
    pass 2 pairs the leftovers (one extra row membership per node); the
    remaining edges ride a per-node [x_n | 0] row.

    Returns slot arrays (s_rl = dst local row; one per slot since both
    halves share the dst), s_row (core-local row id) and the row table
    (rows_node [R,2] with n_local = zero pad; rows_t for window dealing).
    """
    budget = np.full(n_local, BUD, dtype=np.int8)
    bud2 = np.full(n_local, 2, dtype=np.int8)
    zrow = np.full(n_local, -1, dtype=np.int64)

    order = np.argsort(rlk, kind="stable")
    ns, rs = nk[order], rlk[order]
    M = len(ns)
    rows_a, rows_b, rows_t = [], [], []
    s_row, s_rl = [], []

    i = 0
    while i < M:
        j = i
        while j < M and rs[j] == rs[i]:
            j += 1
        d = int(rs[i])
        ti = d // P
        members = [int(n) for n in ns[i:j]]
        i = j
        # pass 1
        lv = []
        pend = None
        for n in members:
            if budget[n] <= 0:
                lv.append(n)
                continue
            if pend is None:
                pend = n
                continue
            r = len(rows_a)
            rows_a.append(pend)
            rows_b.append(n)
            rows_t.append(ti)
            budget[pend] -= 1
            budget[n] -= 1
            s_row.append(r)
            s_rl.append(d)
            pend = None
        if pend is not None:
            lv.append(pend)
        # pass 2 (one extra membership per node)
        lv2 = []
        pend = None
        for n in lv:
            if bud2[n] <= 0:
                lv2.append(n)
                continue
            if pend is None:
                pend = n
                continue
            r = len(rows_a)
            rows_a.append(pend)
            rows_b.append(n)
            rows_t.append(ti)
            bud2[pend] -= 1
            bud2[n] -= 1
            s_row.append(r)
            s_rl.append(d)
            pend = None
        if pend is not None:
            lv2.append(pend)
        # leftovers: per-node shared [x_n | 0] row
        for n in lv2:
            if zrow[n] < 0:
                zrow[n] = len(rows_a)
                rows_a.append(n)
                rows_b.append(n_local)
                rows_t.append(ti)
            s_row.append(int(zrow[n]))
            s_rl.append(d)

    rows_node = np.stack(
        [np.asarray(rows_a, dtype=np.int64), np.asarray(rows_b, dtype=np.int64)],
        axis=1,
    )
    return (np.asarray(s_row, dtype=np.int64), np.asarray(s_rl, dtype=np.int64),
            rows_node, np.asarray(rows_t, dtype=np.int64))


def _plan(row, col, n_local, n_owned):
    """Host-side graph partitioning + pairing. Returns the SPMD-uniform
    schedule and the per-core packed tables."""
    own = n_owned // N_CORES
    T = -(-own // P)                    # dst tiles per core
    S = -(-T // ST)                     # supertiles per core

    row = np.asarray(row).astype(np.int64)
    col = np.asarray(col).astype(np.int64)
    keep = row < n_owned
    r, c = row[keep], col[keep]
    core_of = r // own
    rl_of = r - core_of * own

    per_core_raw = []
    cnt = np.zeros((N_CORES, T, NW), dtype=np.int64)
    for k in range(N_CORES):
        m = core_of == k
        s_row, s_rl, rows_node, rows_t = _pair_core(c[m], rl_of[m], n_local)
        # deal rows round-robin into windows per tile (balances both the
        # int16 windows and the per-(tile,window) slot counts)
        R = len(rows_node)
        order = np.argsort(rows_t, kind="stable")
        alt = np.zeros(R, dtype=np.int64)
        tt = rows_t[order]
        first = np.concatenate(([True], tt[1:] != tt[:-1]))
        grp_start = np.flatnonzero(first)
        pos = np.arange(R) - np.repeat(grp_start, np.diff(
            np.concatenate((grp_start, [R]))))
        alt[order] = pos % NW
        row_gidx = np.empty(R, dtype=np.int64)
        for wbkt in range(NW):
            bw = np.flatnonzero(alt == wbkt)
            assert len(bw) <= WINROWS, (wbkt, len(bw))
            row_gidx[bw] = wbkt * WINROWS + np.arange(len(bw))
        s_g = row_gidx[s_row]
        s_w = s_g // WINROWS
        s_t = s_rl // P
        np.add.at(cnt, (k, s_t, s_w), 1)
        per_core_raw.append((s_t, s_w, s_g - s_w * WINROWS, s_rl,
                             rows_node, row_gidx))

    C = cnt.max(axis=0)  # [T, NW]
    assert C.sum(axis=1).min() > 0, "empty dst tile"

    # geometry: supertile-major, then window, then tile; runs pack
    # UNALIGNED; chunks spanning tile boundaries issue one matmul per
    # covered tile with the one-hot built against an offset iota
    runs = []   # (s, w, t, slot_off)
    calls = []  # (s, w, slot_off, L) per (s,w), L 128-aligned
    jobs = {}   # (s, w) -> list of (chunk_local, tile, iota_variant)
    first_job = {}
    last_job = {}
    off = 0
    t0_of_chunk = []
    for s in range(S):
        ts = range(s * ST, min((s + 1) * ST, T))
        for wi in range(NW):
            call_off = off
            run_list = []
            for ti in ts:
                ln = int(C[ti, wi])
                if ln:
                    runs.append((s, wi, ti, off))
                    run_list.append((ti, off, ln))
                    off += ln
            off_real = off
            L = -(-(off - call_off) // P) * P
            if not L:
                continue
            off = call_off + L
            calls.append((s, wi, call_off, L))
            nchk = L // P
            t0 = [None] * nchk
            jlist = []
            for (ti, o_r, ln) in run_list:
                k_first = (o_r - call_off) // P
                k_last = (o_r + ln - 1 - call_off) // P
                for kc in range(k_first, k_last + 1):
                    if t0[kc] is None:
                        t0[kc] = ti
                    jlist.append((kc, ti))
            jlist.sort()
            jlist2 = []
            for (kc, ti) in jlist:
                v = ti - t0[kc]
                assert 0 <= v < 64
                jlist2.append((kc, ti, v))
                jk = (wi, kc, ti)
                if ti not in first_job:
                    first_job[ti] = jk
                last_job[ti] = jk
            jobs[(s, wi)] = jlist2
            t0_of_chunk.extend(ti if ti is not None else 0 for ti in t0)
    TOT = off
    NCHK = TOT // P
    T0g = np.asarray(t0_of_chunk, dtype=np.int64)
    assert len(T0g) == NCHK

    # per-core slot packing into the uniform geometry
    per_core = []
    lut = np.full(T * NW, -1, dtype=np.int64)
    for (si, wi, ti, o) in runs:
        lut[ti * NW + wi] = o
    for k in range(N_CORES):
        (s_t, s_w, s_gi, s_rl, rows_node, row_gidx) = per_core_raw[k]
        M = len(s_t)
        key = s_t * NW + s_w
        order = np.argsort(key, kind="stable")
        ks = key[order]
        cnts = np.bincount(key, minlength=T * NW)
        starts = np.concatenate(([0], np.cumsum(cnts)))[:-1]
        rank = np.empty(M, dtype=np.int64)
        rank[order] = np.arange(M) - starts[ks]
        dest = lut[key] + rank
        assert dest.min() >= 0 and dest.max() < TOT

        gidx = np.zeros(TOT, dtype=np.int16)
        rl = np.full(TOT, PAD_ROWLOC, dtype=np.float32)
        gidx[dest] = s_gi.astype(np.int16)
        # rowloc relative to the chunk's FIRST covered tile
        rl[dest] = (s_rl - s_t * P) + P * (s_t - T0g[dest // P])

        idx_parts = []
        for (_, _, o, L) in calls:
            for a in range(0, L, GMAX):
                b = min(a + GMAX, L)
                idx_parts.append(_wrap_idx(gidx[o + a:o + b]))
        idx_wrapped = np.concatenate(idx_parts, axis=1)
        RL = np.ascontiguousarray(rl.reshape(NCHK, P).T)
        per_core.append((idx_wrapped, RL, rows_node, row_gidx))

    plan = dict(
        n_local=n_local, n_owned=n_owned, own=own, T=T, S=S,
        C=C, runs=runs, calls=calls, jobs=jobs, TOT=TOT,
        NCHK=NCHK, first_job=first_job, last_job=last_job,
    )
    return plan, per_core


def _build(plan):
    """Emit the Bass/Tile program (identical for all cores)."""
    T, S = plan["T"], plan["S"]
    calls, jobs = plan["calls"], plan["jobs"]
    TOT, NCHK = plan["TOT"], plan["NCHK"]
    first_job, last_job = plan["first_job"], plan["last_job"]

    nc = bacc.Bacc("TRN2", target_bir_lowering=False, debug=False,
                   enable_asserts=False, num_devices=N_CORES)

    xst = nc.dram_tensor("xst", [ROWS, 2 * P], BF16, kind="ExternalInput")
    wgt = nc.dram_tensor("wgt", [P, P], BF16, kind="ExternalInput")
    iot = nc.dram_tensor("iot", [P, P], BF16, kind="ExternalInput")
    idm = nc.dram_tensor("idm", [P, P], BF16, kind="ExternalInput")
    idx = nc.dram_tensor("idx", [P, TOT // 16], I16, kind="ExternalInput")
    rld = nc.dram_tensor("rl", [P, NCHK], F32, kind="ExternalInput")
    out = nc.dram_tensor("out", [P, T * P], BF16, kind="ExternalOutput")

    with tile.TileContext(nc) as tc:
        with (
            tc.tile_pool(name="const", bufs=1) as constp,
            # one shared PSUM pool: every tile is one full bank (the HW
            # zeroes accumulation state at bank granularity, so each
            # accumulator group must own its bank); transposes and the
            # out matmuls reuse the banks round-robin after evacuation
            tc.tile_pool(name="ps", bufs=8, space="PSUM") as psp,
            tc.tile_pool(name="msgs", bufs=4) as msgsp,
            tc.tile_pool(name="sone", bufs=2) as sonep,
            tc.tile_pool(name="agg", bufs=2) as aggp,
            tc.tile_pool(name="osb", bufs=2) as osb,
        ):
            w_sb = constp.tile([P, P], BF16)
            nc.sync.dma_start(w_sb[:], wgt[:, :])
            iot_sb = constp.tile([P, P], BF16)
            nc.sync.dma_start(iot_sb[:], iot[:, :])
            id_sb = constp.tile([P, P], BF16)
            nc.sync.dma_start(id_sb[:], idm[:, :])
            # idx/rl loaded in per-supertile slices so the first gather's
            # descriptor generation isn't blocked on the full metadata load
            idx_sb = constp.tile([P, TOT // 16], I16)
            rl_sb = constp.tile([P, NCHK], F32)
            s_lo = {}
            s_hi = {}
            for (cs, wi, o, L) in calls:
                s_lo[cs] = min(s_lo.get(cs, o), o)
                s_hi[cs] = max(s_hi.get(cs, o + L), o + L)
            for s in range(S):
                lo, hi = s_lo[s], s_hi[s]
                nc.sync.dma_start(idx_sb[:, lo // 16:hi // 16],
                                  idx[:, lo // 16:hi // 16])
                nc.sync.dma_start(rl_sb[:, lo // P:hi // P],
                                  rld[:, lo // P:hi // P])

            max_chunks = max(L for (_, _, _, L) in calls) // P
            max_jobs = max(len(j) for j in jobs.values())

            for s in range(S):
                ts0 = s * ST
                nts = min(ST, T - ts0)
                # one PSUM bank per dst tile: psum[dst, 0:256] accumulates
                # both halves side by side
                pbs = [psp.tile([P, 2 * P], F32, tag="pb", name=f"pb{s}_{i}")
                       for i in range(nts)]
                for (cs, wi, o, L) in calls:
                    if cs != s:
                        continue
                    nchk = L // P
                    wbase = wi * WINROWS
                    mg = msgsp.tile([P, max_chunks * 2 * P], BF16, tag="mg")
                    mg3 = mg[:, :nchk * 2 * P].rearrange(
                        "p (k f) -> p k f", f=2 * P)
                    for a in range(0, L, GMAX):
                        b = min(a + GMAX, L)
                        nc.gpsimd.dma_gather(
                            mg3[:, a // P:b // P, :],
                            xst[wbase:wbase + WINROWS, :],
                            idx_sb[:, (o + a) // 16:(o + b) // 16],
                            b - a, b - a, 2 * P,
                            single_packet=False,
                        )
                    kbase = o // P
                    S_w = sonep.tile([P, max_jobs * P], BF16, tag="S")
                    for jj, (kc, rt, v) in enumerate(jobs[(s, wi)]):
                        tl = rt - ts0
                        kk = kbase + kc
                        S_t = S_w[:, jj * P:(jj + 1) * P]
                        # S = ((iota + 128*v) == rowloc); fp32 ALU keeps
                        # offset iota values exact
                        nc.vector.tensor_scalar(
                            S_t, iot_sb[:],
                            float(v * P), rl_sb[:, kk:kk + 1],
                            mybir.AluOpType.add,
                            mybir.AluOpType.is_equal,
                        )
                        nc.tensor.matmul(
                            pbs[tl][:],
                            S_t, mg3[:, kc, :],
                            start=(first_job[rt] == (wi, kc, rt)),
                            stop=(last_job[rt] == (wi, kc, rt)),
                        )
                # evacuate psum pairs to SBUF (ACT), fold halves to
                # agg[dst, ic] bf16 (DVE), transpose each tile on PE
                # (identity matmul), evacuate (ACT), then
                # out^T = W^T @ aggT reusing the freed banks
                stg = aggp.tile([P, ST * 2 * P], BF16, tag="stg")
                ag = aggp.tile([P, ST * P], BF16, tag="ag")
                for tl in range(nts):
                    nc.scalar.activation(
                        stg[:, tl * 2 * P:(tl + 1) * 2 * P],
                        pbs[tl][:],
                        mybir.ActivationFunctionType.Copy,
                    )
                    nc.vector.tensor_tensor(
                        ag[:, tl * P:(tl + 1) * P],
                        stg[:, tl * 2 * P:tl * 2 * P + P],
                        stg[:, tl * 2 * P + P:(tl + 1) * 2 * P],
                        mybir.AluOpType.add,
                    )
                agT = aggp.tile([P, ST * P], BF16, tag="agT")
                for tl in range(nts):
                    psT = psp.tile([P, P], BF16, tag="pb", name=f"psT{s}_{tl}")
                    nc.tensor.transpose(
                        psT[:], ag[:, tl * P:(tl + 1) * P], id_sb[:])
                    nc.scalar.activation(
                        agT[:, tl * P:(tl + 1) * P],
                        psT[:],
                        mybir.ActivationFunctionType.Copy,
                    )
                ot = osb.tile([P, ST * P], BF16, tag="ot")
                for j0 in range(0, nts * P, 4 * P):
                    nn = min(4 * P, nts * P - j0)
                    po = psp.tile([P, 4 * P], F32, tag="pb", name=f"po{s}_{j0}")
                    nc.tensor.matmul(
                        po[:, :nn], w_sb[:], agT[:, j0:j0 + nn],
                        start=True, stop=True,
                    )
                    nc.scalar.activation(
                        ot[:, j0:j0 + nn],
                        po[:, :nn],
                        mybir.ActivationFunctionType.Copy,
                    )
                nc.sync.dma_start(
                    out[:, ts0 * P:(ts0 + nts) * P], ot[:, :nts * P])

    nc.compile()
    return nc


def _pack_xst(x, deg, rows_node, row_gidx, n_local):
    """Paired node table: row r = [xs[a] | xs[b]] bf16, placed at its
    assigned window position."""
    xs = np.zeros((n_local + 1, P), dtype=np.float32)
    xs[:n_local] = deg[:, None] * x
    tbl = np.zeros((ROWS, 2 * P), dtype=npbf16)
    xs16 = xs.astype(npbf16)
    tbl[row_gidx, :P] = xs16[rows_node[:, 0]]
    tbl[row_gidx, P:] = xs16[rows_node[:, 1]]
    return np.ascontiguousarray(tbl)


_CACHE = {}


def kernel(x, weight, bias, deg_inv_sqrt, row, col, num_owned,
           _want_trace=False):
    n_local = int(x.shape[0])
    n_owned = int(num_owned)
    x = np.asarray(x, dtype=np.float32)
    weight = np.asarray(weight, dtype=np.float32)
    bias = np.asarray(bias, dtype=np.float32)
    deg = np.asarray(deg_inv_sqrt, dtype=np.float32)

    plan, per_core = _plan(row, col, n_local, n_owned)
    wb = weight.astype(npbf16)
    iot = np.ascontiguousarray(
        np.broadcast_to(np.arange(P, dtype=np.float32), (P, P))).astype(npbf16)
    idm = np.eye(P, dtype=np.float32).astype(npbf16)

    sig = (n_local, n_owned, plan["TOT"], plan["C"].tobytes())
    if sig in _CACHE:
        nc = _CACHE[sig]
    else:
        nc = _build(plan)
        _CACHE[sig] = nc

    in_maps = []
    for k in range(N_CORES):
        idxk, rlk, rows_node, row_gidx = per_core[k]
        in_maps.append(dict(
            xst=_pack_xst(x, deg, rows_node, row_gidx, n_local),
            wgt=wb, iot=iot, idm=idm,
            idx=np.ascontiguousarray(idxk), rl=rlk,
        ))

    res = run_bass_kernel_spmd(nc, in_maps, core_ids=list(range(N_CORES)),
                               trace=_want_trace)

    own, T = plan["own"], plan["T"]
    full = np.empty((n_owned, P), dtype=np.float32)
    for k in range(N_CORES):
        outT = np.asarray(res.results[k]["out"], dtype=np.float32)  # [P, T*P]
        full[k * own:(k + 1) * own] = outT.T[:own]
    full *= deg[:n_owned, None]
    full += bias
    kernel.last_results = res
    return full
